# revision 47
# baseline (speedup 1.0000x reference)
"""Segment-mean (sorted index) Trainium2 Bass kernel — v3.

Algorithm (per core, data-parallel over elements, 8 cores; core c owns the
contiguous segment band [base0_c, base0_{c+1})):
  - Host rebases each core's sorted indices to shard-local segment ids
    rel = index - base0_c (< 16384, exact in int16) and ships them packed as
    int16 — halving index HBM traffic.  x ships as float32.
  - Core layout: 128 partitions x (E/128) contiguous elements; each partition
    holds rpp rows of 256.  Heads h[r] = rel[256*r] advance by 0 or 1 per row
    (host-verified), so each row spans at most 2 segments.
  - Phase A (streaming): per 16-row chunk with mid-row base cb = H[mid]:
        xh = fp16(x), dh = fp16(rel - cb)     [Scalar engine]
        ph = dh * xh                          [DVE fp16 2x]
    then per-row sums RS=sum(xh), IXS=sum(ph), SIG=sum(dh) via fp16
    half-fold trees (DVE 2x tensor_tensor adds) + one short tensor_reduce.
    fp16 keeps counts exact: |d| <= 8 so |sum d| <= 2048.
  - Phase B: per-row tail quantities TS = IXS - hp*RS, TC = SIG - 256*hp
    (hp = H - cb, computed on Scalar); runs of equal-head rows -> segmented
    scans; per-partition gpsimd local_scatter places run records at the
    statically aligned slot s = h - K*p + OFS of a 256-wide window (alignment
    host-verified); partition-seam corrections + core-tail ride as extra
    records.  Windows are folded via a DRAM round trip into accA/accC [P, K]
    = per-segment (sum, count) for relative segments K*p + k.
  - No collective: each core writes [accA | accC | mean] as one band.
    kernel() assembles the full [nseg] output on host: band c covers
    [base0_c, base0_{c+1}); the single possibly-shared seam segment
    base0_{c+1} is recombined from both cores' raw (sum, count).
"""

import sys

sys.path.insert(0, "/opt/trn_rl_repo")

import numpy as np

from concourse import bacc, bass, mybir
from concourse import tile
from concourse.bass_utils import run_bass_kernel_spmd

F32 = mybir.dt.float32
F16 = mybir.dt.float16
I32 = mybir.dt.int32
I16 = mybir.dt.int16
U16 = mybir.dt.uint16

AX = mybir.AxisListType.X
OP = mybir.AluOpType

N_CORES = 8
P = 128
ROW = 256
NSEG = 100000
WIN = 256  # scatter window cells per partition
K = 98
OFS = 80
BAND = K * P  # 12544 segments per core band


def build_nc(epc: int):
    """Build the per-core bass program. epc = P * rpp * ROW elements."""
    assert epc % (P * ROW) == 0
    epp = epc // P
    rpp = epp // ROW

    # fold geometry (window -> K-wide per-partition strips)
    m_lo = -((WIN - OFS - 1) // K)
    m_hi = (OFS + K - 1) // K
    pitch = max(OFS - m_lo * K + K, WIN + (m_hi * K - OFS))
    pitch = ((pitch + 31) // 32) * 32
    mpad = max(-m_lo, m_hi) + 1
    wf_rows = ((P + 2 * mpad + 3) // 4) * 4  # x4 so wf_rows*pitch % P == 0

    nc = bacc.Bacc("TRN2", target_bir_lowering=False, debug=False, num_devices=N_CORES)

    idx_ext = nc.declare_dram_parameter("idx", [epc], I16, isOutput=False)
    x_ext = nc.declare_dram_parameter("x", [epc], F32, isOutput=False)
    band_ext = nc.declare_dram_parameter("band", [P * 3 * K], F32, isOutput=True)

    x_v = x_ext.ap().rearrange("(p e) -> p e", p=P)
    i_v = idx_ext.ap().rearrange("(p e) -> p e", p=P)

    # chunk schedule: small ramp, then 16-row chunks
    segs = [(0, 2), (2, 2), (4, 4), (8, 8)]
    r0 = 16
    while r0 < rpp:
        nr = min(16, rpp - r0)
        segs.append((r0, nr))
        r0 += nr
    NCH = len(segs)

    with tile.TileContext(nc) as tc, nc.allow_low_precision(
        reason="fp16 streams: d exact (<=2048), x quantization ~1e-3 << tol"
    ):
        with (
            tc.tile_pool(name="xs", bufs=2) as xpool,
            tc.tile_pool(name="is_", bufs=2) as ipool,
            tc.tile_pool(name="hs", bufs=2) as hpool,
            tc.tile_pool(name="fd", bufs=1) as fpool,
            tc.tile_pool(name="pers", bufs=1) as pp,
        ):
            H = pp.tile([P, rpp], F32, tag="H")  # row heads (relative, exact)
            ncbs = pp.tile([P, NCH], F32, tag="ncbs")  # -cb per chunk
            # width-64 per-row partial-fold accumulator, 3 streams stacked
            # (stream 0 = xh, 1 = dh, 2 = dh*xh — matches the sd tile layout)
            L2A = pp.tile([P, 3 * rpp * 64], F16, tag="L2A")
            RIX = pp.tile([P, 3 * rpp], F32, tag="RIX")  # row sums, 3 streams
            RSf = RIX[:, 0 : rpp]
            SGf = RIX[:, rpp : 2 * rpp]
            IXf = RIX[:, 2 * rpp : 3 * rpp]

            # (K*p - OFS) per-partition constant
            Kp = pp.tile([P, 1], I32, tag="Kp")
            nc.gpsimd.iota(Kp[:], pattern=[[0, 1]], base=0, channel_multiplier=K)
            sbase = pp.tile([P, 1], F32, tag="sbase")
            nc.vector.tensor_scalar(
                out=sbase[:], in0=Kp[:], scalar1=float(-OFS), scalar2=None, op0=OP.add
            )

            Hnf = pp.tile([P, 1], F32, tag="Hnf")
            vmask = pp.tile([P, 1], F32, tag="vmask")

            # tail tiles initialized up front, off the Phase-B critical path
            prevC = pp.tile([P, 3], F32, tag="prevC")
            prevA = pp.tile([P, 2], F32, tag="prevA")
            nc.vector.memset(prevC[:], -999.0)
            nc.vector.memset(prevA[:], 0.0)
            shUp0 = pp.tile([P, K], F32, tag="shUp0")
            shDn0 = pp.tile([P, K], F32, tag="shDn0")
            shUp1 = pp.tile([P, K], F32, tag="shUp1")
            shDn1 = pp.tile([P, K], F32, tag="shDn1")
            shT = {0: (shUp0, shDn0), 1: (shUp1, shDn1)}
            for wi in (0, 1):
                nc.vector.memset(shT[wi][0][:], 0)
                nc.vector.memset(shT[wi][1][0:1, :], 0)

            # pinned ramp tiles: rows [0, 16) arrive via one upfront DMA so the
            # rotating pools are free for the 16-row chunks from t=0
            NRMAX = 16
            SFMAX = NRMAX * ROW
            xr = pp.tile([P, SFMAX], F32, tag="xr")
            nc.sync.dma_start(out=xr[:], in_=x_v[:, 0:SFMAX])

            # ---------------- Phase A: stream chunks ----------------
            for ci, (r0, nr) in enumerate(segs):
                sf = nr * ROW
                cs = slice(r0, r0 + nr)
                mid = r0 + nr // 2
                e0 = r0 * ROW
                it = ipool.tile([P, SFMAX], I16, tag="i")
                nc.gpsimd.dma_start(out=it[:, 0:sf], in_=i_v[:, e0 : e0 + sf])
                if r0 < NRMAX:  # ramp chunk: x comes from the pinned ramp tile
                    c0 = e0
                    xt = xr
                else:
                    c0 = 0
                    xt = xpool.tile([P, SFMAX], F32, tag="x")
                    xq = nc.sync if ci % 2 == 0 else nc.scalar
                    xq.dma_start(out=xt[:, 0:sf], in_=x_v[:, e0 : e0 + sf])

                i3 = it[:, 0:sf].rearrange("p (r e) -> p r e", e=ROW)

                # Scalar: head extraction (strided copy i16->f32), -cb, fp16 conv
                nc.scalar.copy(out=H[:, cs], in_=i3[:, :, 0:1].squeeze(axis=2))
                nc.scalar.mul(
                    out=ncbs[:, ci : ci + 1], in_=H[:, mid : mid + 1], mul=-1.0
                )
                # combined stream tile: [xh | dh | ph] stacked along free dim
                sd = hpool.tile([P, 3 * SFMAX], F16, tag="sd")
                S1, S2 = SFMAX, 2 * SFMAX
                nc.scalar.activation(
                    out=sd[:, 0:sf], in_=xt[:, c0 : c0 + sf],
                    func=mybir.ActivationFunctionType.Copy,
                )
                nc.scalar.activation(
                    out=sd[:, S1 : S1 + sf], in_=it[:, 0:sf],
                    func=mybir.ActivationFunctionType.Identity,
                    bias=ncbs[:, ci : ci + 1], scale=1.0,
                )
                # DVE: products (fp16 2x)
                nc.vector.tensor_tensor(
                    out=sd[:, S2 : S2 + sf], in0=sd[:, S1 : S1 + sf],
                    in1=sd[:, 0:sf], op=OP.mult,
                )

                if r0 == 2:  # after first chunk: Hnf via partition-shift DMA
                    nc.vector.memset(Hnf[:], -1.0)
                    nc.sync.dma_start(out=Hnf[0 : P - 1, :], in_=H[1:P, 0:1])
                    nc.vector.tensor_scalar(
                        out=vmask[:], in0=Hnf[:], scalar1=-1.0, scalar2=None,
                        op0=OP.is_equal,
                    )

                # per-chunk folds: within-row 256 -> 128 -> 64 (fp16 2x),
                # all 3 streams in one wide op per level
                s4 = sd[:].rearrange("p (s q) -> p s q", s=3)[:, :, 0:sf].rearrange(
                    "p s (r e) -> p s r e", e=ROW
                )
                l1 = fpool.tile([P, 3 * NRMAX * 128], F16, tag="l1")
                l14 = l1[:, 0 : 3 * nr * 128].rearrange(
                    "p (s r e) -> p s r e", s=3, e=128
                )
                nc.vector.tensor_tensor(
                    out=l14, in0=s4[:, :, :, 0:128], in1=s4[:, :, :, 128:256],
                    op=OP.add,
                )
                a4 = L2A[:].rearrange("p (s r e) -> p s r e", s=3, e=64)[
                    :, :, r0 : r0 + nr, :
                ]
                nc.vector.tensor_tensor(
                    out=a4, in0=l14[:, :, :, 0:64], in1=l14[:, :, :, 64:128],
                    op=OP.add,
                )

                # half-core fold chains 64 -> 1, emitted mid-loop so they
                # execute inside DMA-wait gaps of the remaining stream
                if r0 + nr in (rpp // 2, rpp):
                    h0 = 0 if r0 + nr == rpp // 2 else rpp // 2
                    hr = rpp // 2
                    c4 = L2A[:].rearrange("p (s r e) -> p s r e", s=3, e=64)[
                        :, :, h0 : h0 + hr, :
                    ]
                    w = 64
                    while w > 2:
                        ctag = "l1" if w == 64 else f"c{w // 2}"
                        nxt = fpool.tile([P, 3 * hr * (w // 2)], F16, tag=ctag)
                        n4 = nxt[:].rearrange("p (s r e) -> p s r e", s=3, e=w // 2)
                        nc.vector.tensor_tensor(
                            out=n4, in0=c4[:, :, :, 0 : w // 2],
                            in1=c4[:, :, :, w // 2 : w], op=OP.add,
                        )
                        c4, w = n4, w // 2
                    r3 = RIX[:].rearrange("p (s r) -> p s r", s=3)
                    nc.vector.tensor_tensor(
                        out=r3[:, :, h0 : h0 + hr],
                        in0=c4[:, :, :, 0:1].squeeze(axis=3),
                        in1=c4[:, :, :, 1:2].squeeze(axis=3), op=OP.add,
                    )

            # ---------------- Phase B ----------------
            # hp = H - cb (per chunk, on Scalar)
            hp = pp.tile([P, rpp], F32, tag="hp")
            for ci, (r0, nr) in enumerate(segs):
                cs = slice(r0, r0 + nr)
                nc.scalar.activation(
                    out=hp[:, cs], in_=H[:, cs],
                    func=mybir.ActivationFunctionType.Identity,
                    bias=ncbs[:, ci : ci + 1], scale=1.0,
                )

            # ---- H-only prep: run flags, slots, scatter indices ----
            same = pp.tile([P, rpp], F32, tag="same")
            nots = pp.tile([P, rpp], F32, tag="nots")
            nc.vector.memset(same[:, 0:1], 0)
            nc.vector.memset(nots[:, 0:1], 0)
            nc.vector.tensor_tensor(
                out=same[:, 1:], in0=H[:, 1:], in1=H[:, :-1], op=OP.is_equal
            )
            nc.vector.tensor_tensor(
                out=nots[:, 1:], in0=H[:, 1:], in1=H[:, :-1], op=OP.not_equal
            )
            lastm = pp.tile([P, rpp], F32, tag="lastm")
            nc.vector.tensor_tensor(
                out=lastm[:, : rpp - 1], in0=H[:, : rpp - 1], in1=H[:, 1:],
                op=OP.not_equal,
            )
            nc.vector.tensor_tensor(
                out=lastm[:, rpp - 1 : rpp], in0=H[:, rpp - 1 : rpp], in1=Hnf[:],
                op=OP.not_equal,
            )
            # slot = H - K*p + OFS
            slotf = pp.tile([P, rpp], F32, tag="slotf")
            nc.vector.tensor_tensor(
                out=slotf[:], in0=H[:],
                in1=sbase[:].to_broadcast([P, rpp]), op=OP.subtract,
            )
            # idxA = lastm ? slot : -1 ; u16-pair indices
            idxAf = pp.tile([P, rpp], F32, tag="idxAf")
            nc.vector.tensor_scalar(
                out=idxAf[:], in0=slotf[:], scalar1=1.0, scalar2=None, op0=OP.add
            )
            nc.vector.tensor_tensor(out=idxAf[:], in0=idxAf[:], in1=lastm[:], op=OP.mult)
            nc.vector.tensor_scalar(
                out=idxAf[:], in0=idxAf[:], scalar1=-1.0, scalar2=None, op0=OP.add
            )
            pidxf = pp.tile([P, 2 * rpp], F32, tag="pidxf")
            p3 = pidxf[:].rearrange("p (r w) -> p r w", w=2)
            t2 = pp.tile([P, rpp], F32, tag="t2")
            nc.vector.tensor_scalar(
                out=t2[:], in0=idxAf[:], scalar1=2.0, scalar2=None, op0=OP.mult
            )
            nc.vector.tensor_copy(out=p3[:, :, 0:1].squeeze(axis=2), in_=t2[:])
            nc.vector.tensor_scalar(
                out=t2[:], in0=t2[:], scalar1=1.0, scalar2=None, op0=OP.add
            )
            nc.vector.tensor_copy(out=p3[:, :, 1:2].squeeze(axis=2), in_=t2[:])
            pidx16 = pp.tile([P, 2 * rpp], I16, tag="pidx16")
            nc.vector.tensor_copy(out=pidx16[:], in_=pidxf[:])
            # extra records: [corr at slot(H[p,0]) (all p), core-tail at
            # slot(H[p,last])+1 (p=127 only, via Hnf sentinel mask)]
            pidxTf = pp.tile([P, 4], F32, tag="pidxTf")
            u2 = pp.tile([P, 1], F32, tag="u2")
            nc.vector.tensor_scalar(
                out=u2[:], in0=slotf[:, 0:1], scalar1=2.0, scalar2=None, op0=OP.mult
            )
            nc.vector.tensor_copy(out=pidxTf[:, 0:1], in_=u2[:])
            nc.vector.tensor_scalar(
                out=pidxTf[:, 1:2], in0=u2[:], scalar1=1.0, scalar2=None, op0=OP.add
            )
            nc.vector.tensor_scalar(
                out=u2[:], in0=slotf[:, rpp - 1 : rpp],
                scalar1=2.0, scalar2=2.0, op0=OP.mult, op1=OP.add,
            )
            nc.vector.tensor_copy(out=pidxTf[:, 2:3], in_=u2[:])
            nc.vector.tensor_scalar(
                out=pidxTf[:, 3:4], in0=u2[:], scalar1=1.0, scalar2=None, op0=OP.add
            )
            nc.vector.tensor_scalar(
                out=pidxTf[:, 2:4], in0=pidxTf[:, 2:4], scalar1=1.0, scalar2=None,
                op0=OP.add,
            )
            nc.vector.tensor_tensor(
                out=pidxTf[:, 2:4], in0=pidxTf[:, 2:4],
                in1=vmask[:].to_broadcast([P, 2]), op=OP.mult,
            )
            nc.vector.tensor_scalar(
                out=pidxTf[:, 2:4], in0=pidxTf[:, 2:4], scalar1=-1.0, scalar2=None,
                op0=OP.add,
            )
            pidxT16 = pp.tile([P, 4], I16, tag="pidxT16")
            nc.vector.tensor_copy(out=pidxT16[:], in_=pidxTf[:])

            bandout = pp.tile([P, 3 * K], F32, tag="bandout")
            accA = bandout[:, 0:K]
            accC = bandout[:, K : 2 * K]
            meanb = bandout[:, 2 * K : 3 * K]
            assert m_lo == -1 and m_hi == 1 and OFS + 2 * K <= pitch

            def win_fold(win, acc, wi):
                # acc[p,k] = win[p][OFS+k] + win[p+1][OFS-K+k] + win[p-1][OFS+K+k]
                shUp, shDn = shT[wi]
                nc.sync.dma_start(
                    out=shUp[0 : P - 1, K - OFS : K], in_=win[1:P, 0:OFS]
                )
                nc.sync.dma_start(
                    out=shDn[1:P, :], in_=win[0 : P - 1, OFS + K : OFS + 2 * K]
                )
                nc.vector.tensor_tensor(
                    out=acc, in0=win[:, OFS : OFS + K], in1=shUp[:], op=OP.add
                )
                nc.vector.tensor_tensor(out=acc, in0=acc, in1=shDn[:], op=OP.add)

            # ================= count path =================
            t256 = pp.tile([P, rpp], F32, tag="t256")
            TCf = pp.tile([P, rpp], F32, tag="TCf")
            nc.vector.tensor_scalar(
                out=t256[:], in0=hp[:], scalar1=float(ROW), scalar2=None, op0=OP.mult
            )
            nc.vector.tensor_tensor(out=TCf[:], in0=SGf, in1=t256[:], op=OP.subtract)
            dataC = pp.tile([P, rpp], F32, tag="dataC")
            inj = pp.tile([P, rpp], F32, tag="inj")
            nc.vector.tensor_scalar(
                out=dataC[:], in0=TCf[:], scalar1=-1.0, scalar2=float(ROW),
                op0=OP.mult, op1=OP.add,
            )
            nc.vector.memset(inj[:, 0:1], 0)
            nc.vector.tensor_tensor(
                out=inj[:, 1:], in0=nots[:, 1:], in1=TCf[:, :-1], op=OP.mult
            )
            nc.vector.tensor_tensor(out=dataC[:], in0=dataC[:], in1=inj[:], op=OP.add)
            scanC = pp.tile([P, rpp], F32, tag="scanC")
            nc.vector.tensor_tensor_scan(
                out=scanC[:], data0=same[:], data1=dataC[:], initial=0.0,
                op0=OP.mult, op1=OP.add,
            )
            winC = pp.tile([P, pitch], F32, tag="winC")
            nc.gpsimd.local_scatter(
                out_ap=winC[:].bitcast(U16), data_ap=scanC[:].bitcast(U16),
                idxs_ap=pidx16[:, 0 : 2 * rpp],
                channels=P, num_elems=2 * pitch, num_idxs=2 * rpp,
            )
            # bounce C: prev partition's [H_last, scanC_last, TCf_last]
            stageC = pp.tile([P, 3], F32, tag="stageC")
            nc.vector.tensor_copy(out=stageC[:, 0:1], in_=H[:, rpp - 1 : rpp])
            nc.vector.tensor_copy(out=stageC[:, 1:2], in_=scanC[:, rpp - 1 : rpp])
            nc.vector.tensor_copy(out=stageC[:, 2:3], in_=TCf[:, rpp - 1 : rpp])
            nc.sync.dma_start(out=prevC[1:P, :], in_=stageC[0 : P - 1, :])
            # cont/tailc flags (shared with sum path)
            h0f = pp.tile([P, 1], F32, tag="h0f")
            cont = pp.tile([P, 1], F32, tag="cont")
            tailc = pp.tile([P, 1], F32, tag="tailc")
            tmp1 = pp.tile([P, 1], F32, tag="tmp1")
            nc.vector.tensor_copy(out=h0f[:], in_=H[:, 0:1])
            nc.vector.tensor_tensor(
                out=cont[:], in0=h0f[:], in1=prevC[:, 0:1], op=OP.is_equal
            )
            nc.vector.tensor_scalar(
                out=tmp1[:], in0=prevC[:, 0:1], scalar1=1.0, scalar2=None, op0=OP.add
            )
            nc.vector.tensor_tensor(
                out=tailc[:], in0=h0f[:], in1=tmp1[:], op=OP.is_equal
            )
            corrBC = pp.tile([P, 2], F32, tag="corrBC")  # [corrC, TCf_last]
            nc.vector.tensor_tensor(
                out=corrBC[:, 0:1], in0=cont[:], in1=prevC[:, 1:2], op=OP.mult
            )
            nc.vector.tensor_tensor(out=tmp1[:], in0=tailc[:], in1=prevC[:, 2:3], op=OP.mult)
            nc.vector.tensor_tensor(
                out=corrBC[:, 0:1], in0=corrBC[:, 0:1], in1=tmp1[:], op=OP.add
            )
            nc.vector.tensor_copy(out=corrBC[:, 1:2], in_=TCf[:, rpp - 1 : rpp])
            winTC = pp.tile([P, pitch], F32, tag="winTC")
            nc.gpsimd.local_scatter(
                out_ap=winTC[:].bitcast(U16), data_ap=corrBC[:].bitcast(U16),
                idxs_ap=pidxT16[:, 0:4],
                channels=P, num_elems=2 * pitch, num_idxs=4,
            )
            nc.vector.tensor_tensor(out=winC[:], in0=winC[:], in1=winTC[:], op=OP.add)
            win_fold(winC, accC, 1)
            rec = pp.tile([P, K], F32, tag="rec")
            nc.vector.tensor_scalar(
                out=rec[:], in0=accC, scalar1=1.0, scalar2=None, op0=OP.max
            )
            nc.vector.reciprocal(out=rec[:], in_=rec[:])

            # ================= sum path =================
            TS = pp.tile([P, rpp], F32, tag="TS")
            nc.vector.tensor_tensor(out=t256[:], in0=hp[:], in1=RSf, op=OP.mult)
            nc.vector.tensor_tensor(out=TS[:], in0=IXf, in1=t256[:], op=OP.subtract)
            dataA = pp.tile([P, rpp], F32, tag="dataA")
            nc.vector.tensor_tensor(out=dataA[:], in0=RSf, in1=TS[:], op=OP.subtract)
            nc.vector.tensor_tensor(
                out=inj[:, 1:], in0=nots[:, 1:], in1=TS[:, :-1], op=OP.mult
            )
            nc.vector.memset(inj[:, 0:1], 0)
            nc.vector.tensor_tensor(out=dataA[:], in0=dataA[:], in1=inj[:], op=OP.add)
            scanA = pp.tile([P, rpp], F32, tag="scanA")
            nc.vector.tensor_tensor_scan(
                out=scanA[:], data0=same[:], data1=dataA[:], initial=0.0,
                op0=OP.mult, op1=OP.add,
            )
            winA = pp.tile([P, pitch], F32, tag="winA")
            nc.gpsimd.local_scatter(
                out_ap=winA[:].bitcast(U16), data_ap=scanA[:].bitcast(U16),
                idxs_ap=pidx16[:, 0 : 2 * rpp],
                channels=P, num_elems=2 * pitch, num_idxs=2 * rpp,
            )
            # bounce A: prev partition's [scanA_last, TS_last]
            stageA = pp.tile([P, 2], F32, tag="stageA")
            nc.vector.tensor_copy(out=stageA[:, 0:1], in_=scanA[:, rpp - 1 : rpp])
            nc.vector.tensor_copy(out=stageA[:, 1:2], in_=TS[:, rpp - 1 : rpp])
            nc.sync.dma_start(out=prevA[1:P, :], in_=stageA[0 : P - 1, :])
            corrB = pp.tile([P, 2], F32, tag="corrB")  # [corrA, TS_last]
            nc.vector.tensor_tensor(
                out=corrB[:, 0:1], in0=cont[:], in1=prevA[:, 0:1], op=OP.mult
            )
            nc.vector.tensor_tensor(out=tmp1[:], in0=tailc[:], in1=prevA[:, 1:2], op=OP.mult)
            nc.vector.tensor_tensor(
                out=corrB[:, 0:1], in0=corrB[:, 0:1], in1=tmp1[:], op=OP.add
            )
            nc.vector.tensor_copy(out=corrB[:, 1:2], in_=TS[:, rpp - 1 : rpp])
            winT = pp.tile([P, pitch], F32, tag="winT")
            nc.gpsimd.local_scatter(
                out_ap=winT[:].bitcast(U16), data_ap=corrB[:].bitcast(U16),
                idxs_ap=pidxT16[:, 0:4],
                channels=P, num_elems=2 * pitch, num_idxs=4,
            )
            nc.vector.tensor_tensor(out=winA[:], in0=winA[:], in1=winT[:], op=OP.add)
            win_fold(winA, accA, 0)
            bv = band_ext.ap().rearrange("(p k) -> p k", p=P)
            nc.sync.dma_start(out=bv[:, 0 : 2 * K], in_=bandout[:, 0 : 2 * K])
            nc.vector.tensor_tensor(out=meanb, in0=accA, in1=rec[:], op=OP.mult)
            nc.sync.dma_start(out=bv[:, 2 * K : 3 * K], in_=meanb)

    nc.finalize()
    return nc


_NC_CACHE: dict = {}


def _get_nc(*key):
    if key not in _NC_CACHE:
        _NC_CACHE[key] = build_nc(*key)
    return _NC_CACHE[key]


def kernel(x: np.ndarray, index: np.ndarray) -> np.ndarray:
    n = x.shape[0]
    assert n % (N_CORES * P * ROW) == 0, n
    epc = n // N_CORES

    # cheap structural checks on row heads (the algorithm's contract)
    heads = np.ascontiguousarray(index[::ROW]).astype(np.int64)
    dhh = np.diff(heads)
    if dhh.min() < 0 or dhh.max() > 1:
        raise ValueError("row-head steps outside {0,1}; kernel contract violated")
    hc = heads.reshape(N_CORES, P, -1)
    rel = hc - hc[:, 0:1, 0:1]
    slot = rel - K * np.arange(P)[None, :, None] + OFS
    if slot.min() < 0 or slot.max() + 1 >= WIN:
        raise ValueError("alignment window overflow; adjust K/OFS")
    if rel.max() + 1 >= 16384:
        raise ValueError("relative segment id exceeds int16 range")
    base0s = hc[:, 0, 0].astype(np.int64)  # first segment of each core
    widths = np.diff(np.concatenate([base0s, [NSEG]]))
    if widths.min() < 2 or widths.max() > BAND:
        raise ValueError("band widths outside (2, BAND]; kernel contract violated")

    nc = _get_nc(epc)

    in_maps = []
    for c in range(N_CORES):
        xs = np.ascontiguousarray(x[c * epc : (c + 1) * epc], dtype=np.float32)
        ii = (index[c * epc : (c + 1) * epc] - base0s[c]).astype(np.int16)
        in_maps.append({"x": xs, "idx": ii})

    res = run_bass_kernel_spmd(
        nc, in_maps, core_ids=list(range(N_CORES)), trace=TRACE, **RUN_KWARGS
    )
    global LAST_RESULT
    LAST_RESULT = res

    # host gather/unshard: concatenate per-core bands; recombine seam segments
    out = np.zeros(NSEG, dtype=np.float32)
    sums, cnts, means = [], [], []
    for c in range(N_CORES):
        arr = np.asarray(res.results[c]["band"], dtype=np.float32).reshape(P, 3 * K)
        sums.append(arr[:, 0:K].ravel())
        cnts.append(arr[:, K : 2 * K].ravel())
        means.append(arr[:, 2 * K : 3 * K].ravel())
    for c in range(N_CORES):
        lo = int(base0s[c])
        hi = int(base0s[c + 1]) if c < N_CORES - 1 else NSEG
        out[lo:hi] = means[c][0 : hi - lo]
    for c in range(N_CORES - 1):
        s = int(base0s[c + 1])  # seam segment shared by cores c and c+1
        if s >= NSEG:
            continue
        d = s - int(base0s[c])
        tot = sums[c][d] + sums[c + 1][0]
        cnt = cnts[c][d] + cnts[c + 1][0]
        out[s] = tot / max(cnt, 1.0)
    return out


TRACE = False
RUN_KWARGS: dict = {}
LAST_RESULT = None


# revision 48
# speedup vs baseline: 1.0421x; 1.0421x over previous
"""Segment-mean (sorted index) Trainium2 Bass kernel — v3.

Algorithm (per core, data-parallel over elements, 8 cores; core c owns the
contiguous segment band [base0_c, base0_{c+1})):
  - Host rebases each core's sorted indices to shard-local segment ids
    rel = index - base0_c (< 16384, exact in int16) and ships them packed as
    int16 — halving index HBM traffic.  x ships as float32.
  - Core layout: 128 partitions x (E/128) contiguous elements; each partition
    holds rpp rows of 256.  Heads h[r] = rel[256*r] advance by 0 or 1 per row
    (host-verified), so each row spans at most 2 segments.
  - Phase A (streaming): per 16-row chunk with mid-row base cb = H[mid]:
        xh = fp16(x), dh = fp16(rel - cb)     [Scalar engine]
        ph = dh * xh                          [DVE fp16 2x]
    then per-row sums RS=sum(xh), IXS=sum(ph), SIG=sum(dh) via fp16
    half-fold trees (DVE 2x tensor_tensor adds) + one short tensor_reduce.
    fp16 keeps counts exact: |d| <= 8 so |sum d| <= 2048.
  - Phase B: per-row tail quantities TS = IXS - hp*RS, TC = SIG - 256*hp
    (hp = H - cb, computed on Scalar); runs of equal-head rows -> segmented
    scans; per-partition gpsimd local_scatter places run records at the
    statically aligned slot s = h - K*p + OFS of a 256-wide window (alignment
    host-verified); partition-seam corrections + core-tail ride as extra
    records.  Windows are folded via a DRAM round trip into accA/accC [P, K]
    = per-segment (sum, count) for relative segments K*p + k.
  - No collective: each core writes [accA | accC | mean] as one band.
    kernel() assembles the full [nseg] output on host: band c covers
    [base0_c, base0_{c+1}); the single possibly-shared seam segment
    base0_{c+1} is recombined from both cores' raw (sum, count).
"""

import sys

sys.path.insert(0, "/opt/trn_rl_repo")

import numpy as np

from concourse import bacc, bass, mybir
from concourse import tile
from concourse.bass_utils import run_bass_kernel_spmd

F32 = mybir.dt.float32
F16 = mybir.dt.float16
I32 = mybir.dt.int32
I16 = mybir.dt.int16
U16 = mybir.dt.uint16

AX = mybir.AxisListType.X
OP = mybir.AluOpType

N_CORES = 8
P = 128
ROW = 256
NSEG = 100000
WIN = 256  # scatter window cells per partition
K = 98
OFS = 80
BAND = K * P  # 12544 segments per core band


def build_nc(epc: int):
    """Build the per-core bass program. epc = P * rpp * ROW elements."""
    assert epc % (P * ROW) == 0
    epp = epc // P
    rpp = epp // ROW

    # fold geometry (window -> K-wide per-partition strips)
    m_lo = -((WIN - OFS - 1) // K)
    m_hi = (OFS + K - 1) // K
    pitch = max(OFS - m_lo * K + K, WIN + (m_hi * K - OFS))
    pitch = ((pitch + 31) // 32) * 32
    mpad = max(-m_lo, m_hi) + 1
    wf_rows = ((P + 2 * mpad + 3) // 4) * 4  # x4 so wf_rows*pitch % P == 0

    nc = bacc.Bacc("TRN2", target_bir_lowering=False, debug=False, num_devices=N_CORES)

    idx_ext = nc.declare_dram_parameter("idx", [epc], I16, isOutput=False)
    x_ext = nc.declare_dram_parameter("x", [epc], F32, isOutput=False)
    band_ext = nc.declare_dram_parameter("band", [P * 3 * K], F32, isOutput=True)

    x_v = x_ext.ap().rearrange("(p e) -> p e", p=P)
    i_v = idx_ext.ap().rearrange("(p e) -> p e", p=P)

    # chunk schedule: small ramp, then 16-row chunks
    segs = [(0, 2), (2, 2), (4, 4), (8, 8)]
    r0 = 16
    while r0 < rpp:
        nr = min(16, rpp - r0)
        segs.append((r0, nr))
        r0 += nr
    NCH = len(segs)

    with tile.TileContext(nc) as tc, nc.allow_low_precision(
        reason="fp16 streams: d exact (<=2048), x quantization ~1e-3 << tol"
    ):
        with (
            tc.tile_pool(name="xs", bufs=2) as xpool,
            tc.tile_pool(name="is_", bufs=2) as ipool,
            tc.tile_pool(name="hs", bufs=2) as hpool,
            tc.tile_pool(name="fd", bufs=1) as fpool,
            tc.tile_pool(name="pers", bufs=1) as pp,
        ):
            H = pp.tile([P, rpp], F32, tag="H")  # row heads (relative, exact)
            ncbs = pp.tile([P, NCH], F32, tag="ncbs")  # -cb per chunk
            # width-64 per-row partial-fold accumulator, 3 streams stacked
            # (stream 0 = xh, 1 = dh, 2 = dh*xh — matches the sd tile layout)
            L2A = pp.tile([P, 3 * rpp * 64], F16, tag="L2A")
            RIX = pp.tile([P, 3 * rpp], F32, tag="RIX")  # row sums, 3 streams
            RSf = RIX[:, 0 : rpp]
            SGf = RIX[:, rpp : 2 * rpp]
            IXf = RIX[:, 2 * rpp : 3 * rpp]

            # (K*p - OFS) per-partition constant
            Kp = pp.tile([P, 1], I32, tag="Kp")
            nc.gpsimd.iota(Kp[:], pattern=[[0, 1]], base=0, channel_multiplier=K)
            sbase = pp.tile([P, 1], F32, tag="sbase")
            nc.vector.tensor_scalar(
                out=sbase[:], in0=Kp[:], scalar1=float(-OFS), scalar2=None, op0=OP.add
            )

            Hnf = pp.tile([P, 1], F32, tag="Hnf")
            vmask = pp.tile([P, 1], F32, tag="vmask")

            # tail tiles initialized up front, off the Phase-B critical path
            prevC = pp.tile([P, 3], F32, tag="prevC")
            prevA = pp.tile([P, 2], F32, tag="prevA")
            nc.vector.memset(prevC[:], -999.0)
            nc.vector.memset(prevA[:], 0.0)
            shUp0 = pp.tile([P, K], F32, tag="shUp0")
            shDn0 = pp.tile([P, K], F32, tag="shDn0")
            shUp1 = pp.tile([P, K], F32, tag="shUp1")
            shDn1 = pp.tile([P, K], F32, tag="shDn1")
            shT = {0: (shUp0, shDn0), 1: (shUp1, shDn1)}
            for wi in (0, 1):
                nc.vector.memset(shT[wi][0][:], 0)
                nc.vector.memset(shT[wi][1][0:1, :], 0)

            # pinned ramp tiles: rows [0, 16) arrive via one upfront DMA so the
            # rotating pools are free for the 16-row chunks from t=0
            NRMAX = 16
            SFMAX = NRMAX * ROW
            xr = pp.tile([P, SFMAX], F32, tag="xr")
            nc.sync.dma_start(out=xr[:], in_=x_v[:, 0:SFMAX])

            # ---------------- Phase A: stream chunks ----------------
            for ci, (r0, nr) in enumerate(segs):
                sf = nr * ROW
                cs = slice(r0, r0 + nr)
                mid = r0 + nr // 2
                e0 = r0 * ROW
                it = ipool.tile([P, SFMAX], I16, tag="i")
                nc.gpsimd.dma_start(out=it[:, 0:sf], in_=i_v[:, e0 : e0 + sf])
                if r0 < NRMAX:  # ramp chunk: x comes from the pinned ramp tile
                    c0 = e0
                    xt = xr
                else:
                    c0 = 0
                    xt = xpool.tile([P, SFMAX], F32, tag="x")
                    xq = nc.sync if ci % 2 == 0 else nc.scalar
                    xq.dma_start(out=xt[:, 0:sf], in_=x_v[:, e0 : e0 + sf])

                i3 = it[:, 0:sf].rearrange("p (r e) -> p r e", e=ROW)

                # Scalar: head extraction (strided copy i16->f32), -cb, fp16 conv
                nc.scalar.copy(out=H[:, cs], in_=i3[:, :, 0:1].squeeze(axis=2))
                nc.scalar.mul(
                    out=ncbs[:, ci : ci + 1], in_=H[:, mid : mid + 1], mul=-1.0
                )
                # combined stream tile: [xh | dh | ph] stacked along free dim
                sd = hpool.tile([P, 3 * SFMAX], F16, tag="sd")
                S1, S2 = SFMAX, 2 * SFMAX
                nc.scalar.activation(
                    out=sd[:, 0:sf], in_=xt[:, c0 : c0 + sf],
                    func=mybir.ActivationFunctionType.Copy,
                )
                nc.scalar.activation(
                    out=sd[:, S1 : S1 + sf], in_=it[:, 0:sf],
                    func=mybir.ActivationFunctionType.Identity,
                    bias=ncbs[:, ci : ci + 1], scale=1.0,
                )
                # DVE: products (fp16 2x)
                nc.vector.tensor_tensor(
                    out=sd[:, S2 : S2 + sf], in0=sd[:, S1 : S1 + sf],
                    in1=sd[:, 0:sf], op=OP.mult,
                )

                if r0 == 2:  # after first chunk: Hnf via partition-shift DMA
                    nc.vector.memset(Hnf[:], -1.0)
                    nc.sync.dma_start(out=Hnf[0 : P - 1, :], in_=H[1:P, 0:1])
                    nc.vector.tensor_scalar(
                        out=vmask[:], in0=Hnf[:], scalar1=-1.0, scalar2=None,
                        op0=OP.is_equal,
                    )

                # per-chunk folds: within-row 256 -> 128 -> 64 (fp16 2x),
                # all 3 streams in one wide op per level
                s4 = sd[:].rearrange("p (s q) -> p s q", s=3)[:, :, 0:sf].rearrange(
                    "p s (r e) -> p s r e", e=ROW
                )
                l1 = fpool.tile([P, 3 * NRMAX * 128], F16, tag="l1")
                l14 = l1[:, 0 : 3 * nr * 128].rearrange(
                    "p (s r e) -> p s r e", s=3, e=128
                )
                nc.vector.tensor_tensor(
                    out=l14, in0=s4[:, :, :, 0:128], in1=s4[:, :, :, 128:256],
                    op=OP.add,
                )
                a4 = L2A[:].rearrange("p (s r e) -> p s r e", s=3, e=64)[
                    :, :, r0 : r0 + nr, :
                ]
                nc.vector.tensor_tensor(
                    out=a4, in0=l14[:, :, :, 0:64], in1=l14[:, :, :, 64:128],
                    op=OP.add,
                )

                # half-core fold chains 64 -> 1, emitted mid-loop so they
                # execute inside DMA-wait gaps of the remaining stream
                if r0 + nr in (rpp // 2, rpp):
                    h0 = 0 if r0 + nr == rpp // 2 else rpp // 2
                    hr = rpp // 2
                    c4 = L2A[:].rearrange("p (s r e) -> p s r e", s=3, e=64)[
                        :, :, h0 : h0 + hr, :
                    ]
                    w = 64
                    while w > 2:
                        ctag = "l1" if w == 64 else f"c{w // 2}"
                        nxt = fpool.tile([P, 3 * hr * (w // 2)], F16, tag=ctag)
                        n4 = nxt[:].rearrange("p (s r e) -> p s r e", s=3, e=w // 2)
                        nc.vector.tensor_tensor(
                            out=n4, in0=c4[:, :, :, 0 : w // 2],
                            in1=c4[:, :, :, w // 2 : w], op=OP.add,
                        )
                        c4, w = n4, w // 2
                    r3 = RIX[:].rearrange("p (s r) -> p s r", s=3)
                    nc.vector.tensor_tensor(
                        out=r3[:, :, h0 : h0 + hr],
                        in0=c4[:, :, :, 0:1].squeeze(axis=3),
                        in1=c4[:, :, :, 1:2].squeeze(axis=3), op=OP.add,
                    )

            # ---------------- Phase B ----------------
            # hp = H - cb (per chunk, on Scalar)
            hp = pp.tile([P, rpp], F32, tag="hp")
            for ci, (r0, nr) in enumerate(segs):
                cs = slice(r0, r0 + nr)
                nc.scalar.activation(
                    out=hp[:, cs], in_=H[:, cs],
                    func=mybir.ActivationFunctionType.Identity,
                    bias=ncbs[:, ci : ci + 1], scale=1.0,
                )

            # ---- H-only prep: run flags, slots, scatter indices ----
            same = pp.tile([P, rpp], F32, tag="same")
            nots = pp.tile([P, rpp], F32, tag="nots")
            nc.vector.memset(same[:, 0:1], 0)
            nc.vector.memset(nots[:, 0:1], 0)
            nc.vector.tensor_tensor(
                out=same[:, 1:], in0=H[:, 1:], in1=H[:, :-1], op=OP.is_equal
            )
            nc.vector.tensor_tensor(
                out=nots[:, 1:], in0=H[:, 1:], in1=H[:, :-1], op=OP.not_equal
            )
            lastm = pp.tile([P, rpp], F32, tag="lastm")
            nc.vector.tensor_tensor(
                out=lastm[:, : rpp - 1], in0=H[:, : rpp - 1], in1=H[:, 1:],
                op=OP.not_equal,
            )
            nc.vector.tensor_tensor(
                out=lastm[:, rpp - 1 : rpp], in0=H[:, rpp - 1 : rpp], in1=Hnf[:],
                op=OP.not_equal,
            )
            # slot = H - K*p + OFS
            slotf = pp.tile([P, rpp], F32, tag="slotf")
            nc.vector.tensor_tensor(
                out=slotf[:], in0=H[:],
                in1=sbase[:].to_broadcast([P, rpp]), op=OP.subtract,
            )
            # idxA = lastm ? slot : -1 ; u16-pair indices
            idxAf = pp.tile([P, rpp], F32, tag="idxAf")
            nc.vector.tensor_scalar(
                out=idxAf[:], in0=slotf[:], scalar1=1.0, scalar2=None, op0=OP.add
            )
            nc.vector.tensor_tensor(out=idxAf[:], in0=idxAf[:], in1=lastm[:], op=OP.mult)
            nc.vector.tensor_scalar(
                out=idxAf[:], in0=idxAf[:], scalar1=-1.0, scalar2=None, op0=OP.add
            )
            pidxf = pp.tile([P, 2 * rpp], F32, tag="pidxf")
            p3 = pidxf[:].rearrange("p (r w) -> p r w", w=2)
            t2 = pp.tile([P, rpp], F32, tag="t2")
            nc.vector.tensor_scalar(
                out=t2[:], in0=idxAf[:], scalar1=2.0, scalar2=None, op0=OP.mult
            )
            nc.vector.tensor_copy(out=p3[:, :, 0:1].squeeze(axis=2), in_=t2[:])
            nc.vector.tensor_scalar(
                out=t2[:], in0=t2[:], scalar1=1.0, scalar2=None, op0=OP.add
            )
            nc.vector.tensor_copy(out=p3[:, :, 1:2].squeeze(axis=2), in_=t2[:])
            pidx16 = pp.tile([P, 2 * rpp], I16, tag="pidx16")
            nc.vector.tensor_copy(out=pidx16[:], in_=pidxf[:])
            # extra records: [corr at slot(H[p,0]) (all p), core-tail at
            # slot(H[p,last])+1 (p=127 only, via Hnf sentinel mask)]
            pidxTf = pp.tile([P, 4], F32, tag="pidxTf")
            u2 = pp.tile([P, 1], F32, tag="u2")
            nc.vector.tensor_scalar(
                out=u2[:], in0=slotf[:, 0:1], scalar1=2.0, scalar2=None, op0=OP.mult
            )
            nc.vector.tensor_copy(out=pidxTf[:, 0:1], in_=u2[:])
            nc.vector.tensor_scalar(
                out=pidxTf[:, 1:2], in0=u2[:], scalar1=1.0, scalar2=None, op0=OP.add
            )
            nc.vector.tensor_scalar(
                out=u2[:], in0=slotf[:, rpp - 1 : rpp],
                scalar1=2.0, scalar2=2.0, op0=OP.mult, op1=OP.add,
            )
            nc.vector.tensor_copy(out=pidxTf[:, 2:3], in_=u2[:])
            nc.vector.tensor_scalar(
                out=pidxTf[:, 3:4], in0=u2[:], scalar1=1.0, scalar2=None, op0=OP.add
            )
            nc.vector.tensor_scalar(
                out=pidxTf[:, 2:4], in0=pidxTf[:, 2:4], scalar1=1.0, scalar2=None,
                op0=OP.add,
            )
            nc.vector.tensor_tensor(
                out=pidxTf[:, 2:4], in0=pidxTf[:, 2:4],
                in1=vmask[:].to_broadcast([P, 2]), op=OP.mult,
            )
            nc.vector.tensor_scalar(
                out=pidxTf[:, 2:4], in0=pidxTf[:, 2:4], scalar1=-1.0, scalar2=None,
                op0=OP.add,
            )
            pidxT16 = pp.tile([P, 4], I16, tag="pidxT16")
            nc.vector.tensor_copy(out=pidxT16[:], in_=pidxTf[:])

            bandout = pp.tile([P, 3 * K], F32, tag="bandout")
            accA = bandout[:, 0:K]
            accC = bandout[:, K : 2 * K]
            meanb = bandout[:, 2 * K : 3 * K]
            assert m_lo == -1 and m_hi == 1 and OFS + 2 * K <= pitch

            def win_fold(win, acc, wi):
                # acc[p,k] = win[p][OFS+k] + win[p+1][OFS-K+k] + win[p-1][OFS+K+k]
                shUp, shDn = shT[wi]
                nc.sync.dma_start(
                    out=shUp[0 : P - 1, K - OFS : K], in_=win[1:P, 0:OFS]
                )
                nc.sync.dma_start(
                    out=shDn[1:P, :], in_=win[0 : P - 1, OFS + K : OFS + 2 * K]
                )
                nc.vector.tensor_tensor(
                    out=acc, in0=win[:, OFS : OFS + K], in1=shUp[:], op=OP.add
                )
                nc.vector.tensor_tensor(out=acc, in0=acc, in1=shDn[:], op=OP.add)

            # ================= count path =================
            t256 = pp.tile([P, rpp], F32, tag="t256")
            TCf = pp.tile([P, rpp], F32, tag="TCf")
            nc.vector.tensor_scalar(
                out=t256[:], in0=hp[:], scalar1=float(ROW), scalar2=None, op0=OP.mult
            )
            nc.vector.tensor_tensor(out=TCf[:], in0=SGf, in1=t256[:], op=OP.subtract)
            dataC = pp.tile([P, rpp], F32, tag="dataC")
            inj = pp.tile([P, rpp], F32, tag="inj")
            nc.vector.tensor_scalar(
                out=dataC[:], in0=TCf[:], scalar1=-1.0, scalar2=float(ROW),
                op0=OP.mult, op1=OP.add,
            )
            nc.vector.memset(inj[:, 0:1], 0)
            nc.vector.tensor_tensor(
                out=inj[:, 1:], in0=nots[:, 1:], in1=TCf[:, :-1], op=OP.mult
            )
            nc.vector.tensor_tensor(out=dataC[:], in0=dataC[:], in1=inj[:], op=OP.add)
            scanC = pp.tile([P, rpp], F32, tag="scanC")
            nc.vector.tensor_tensor_scan(
                out=scanC[:], data0=same[:], data1=dataC[:], initial=0.0,
                op0=OP.mult, op1=OP.add,
            )
            winC = pp.tile([P, pitch], F32, tag="winC")
            nc.gpsimd.local_scatter(
                out_ap=winC[:].bitcast(U16), data_ap=scanC[:].bitcast(U16),
                idxs_ap=pidx16[:, 0 : 2 * rpp],
                channels=P, num_elems=2 * pitch, num_idxs=2 * rpp,
            )
            # bounce C: prev partition's [H_last, scanC_last, TCf_last]
            stageC = pp.tile([P, 3], F32, tag="stageC")
            nc.vector.tensor_copy(out=stageC[:, 0:1], in_=H[:, rpp - 1 : rpp])
            nc.vector.tensor_copy(out=stageC[:, 1:2], in_=scanC[:, rpp - 1 : rpp])
            nc.vector.tensor_copy(out=stageC[:, 2:3], in_=TCf[:, rpp - 1 : rpp])
            nc.sync.dma_start(out=prevC[1:P, :], in_=stageC[0 : P - 1, :])
            # cont/tailc flags (shared with sum path)
            h0f = pp.tile([P, 1], F32, tag="h0f")
            cont = pp.tile([P, 1], F32, tag="cont")
            tailc = pp.tile([P, 1], F32, tag="tailc")
            tmp1 = pp.tile([P, 1], F32, tag="tmp1")
            nc.vector.tensor_copy(out=h0f[:], in_=H[:, 0:1])
            nc.vector.tensor_tensor(
                out=cont[:], in0=h0f[:], in1=prevC[:, 0:1], op=OP.is_equal
            )
            nc.vector.tensor_scalar(
                out=tmp1[:], in0=prevC[:, 0:1], scalar1=1.0, scalar2=None, op0=OP.add
            )
            nc.vector.tensor_tensor(
                out=tailc[:], in0=h0f[:], in1=tmp1[:], op=OP.is_equal
            )
            corrBC = pp.tile([P, 2], F32, tag="corrBC")  # [corrC, TCf_last]
            nc.vector.tensor_tensor(
                out=corrBC[:, 0:1], in0=cont[:], in1=prevC[:, 1:2], op=OP.mult
            )
            nc.vector.tensor_tensor(out=tmp1[:], in0=tailc[:], in1=prevC[:, 2:3], op=OP.mult)
            nc.vector.tensor_tensor(
                out=corrBC[:, 0:1], in0=corrBC[:, 0:1], in1=tmp1[:], op=OP.add
            )
            nc.vector.tensor_copy(out=corrBC[:, 1:2], in_=TCf[:, rpp - 1 : rpp])
            winTC = pp.tile([P, pitch], F32, tag="winTC")
            nc.gpsimd.local_scatter(
                out_ap=winTC[:].bitcast(U16), data_ap=corrBC[:].bitcast(U16),
                idxs_ap=pidxT16[:, 0:4],
                channels=P, num_elems=2 * pitch, num_idxs=4,
            )
            nc.vector.tensor_tensor(out=winC[:], in0=winC[:], in1=winTC[:], op=OP.add)
            win_fold(winC, accC, 1)
            rec = pp.tile([P, K], F32, tag="rec")
            nc.vector.tensor_scalar(
                out=rec[:], in0=accC, scalar1=1.0, scalar2=None, op0=OP.max
            )
            nc.vector.reciprocal(out=rec[:], in_=rec[:])

            # ================= sum path =================
            TS = pp.tile([P, rpp], F32, tag="TS")
            nc.vector.tensor_tensor(out=t256[:], in0=hp[:], in1=RSf, op=OP.mult)
            nc.vector.tensor_tensor(out=TS[:], in0=IXf, in1=t256[:], op=OP.subtract)
            dataA = pp.tile([P, rpp], F32, tag="dataA")
            nc.vector.tensor_tensor(out=dataA[:], in0=RSf, in1=TS[:], op=OP.subtract)
            nc.vector.tensor_tensor(
                out=inj[:, 1:], in0=nots[:, 1:], in1=TS[:, :-1], op=OP.mult
            )
            nc.vector.memset(inj[:, 0:1], 0)
            nc.vector.tensor_tensor(out=dataA[:], in0=dataA[:], in1=inj[:], op=OP.add)
            scanA = pp.tile([P, rpp], F32, tag="scanA")
            nc.vector.tensor_tensor_scan(
                out=scanA[:], data0=same[:], data1=dataA[:], initial=0.0,
                op0=OP.mult, op1=OP.add,
            )
            winA = pp.tile([P, pitch], F32, tag="winA")
            nc.gpsimd.local_scatter(
                out_ap=winA[:].bitcast(U16), data_ap=scanA[:].bitcast(U16),
                idxs_ap=pidx16[:, 0 : 2 * rpp],
                channels=P, num_elems=2 * pitch, num_idxs=2 * rpp,
            )
            # bounce A: prev partition's [scanA_last, TS_last]
            stageA = pp.tile([P, 2], F32, tag="stageA")
            nc.vector.tensor_copy(out=stageA[:, 0:1], in_=scanA[:, rpp - 1 : rpp])
            nc.vector.tensor_copy(out=stageA[:, 1:2], in_=TS[:, rpp - 1 : rpp])
            nc.sync.dma_start(out=prevA[1:P, :], in_=stageA[0 : P - 1, :])
            corrB = pp.tile([P, 2], F32, tag="corrB")  # [corrA, TS_last]
            nc.vector.tensor_tensor(
                out=corrB[:, 0:1], in0=cont[:], in1=prevA[:, 0:1], op=OP.mult
            )
            nc.vector.tensor_tensor(out=tmp1[:], in0=tailc[:], in1=prevA[:, 1:2], op=OP.mult)
            nc.vector.tensor_tensor(
                out=corrB[:, 0:1], in0=corrB[:, 0:1], in1=tmp1[:], op=OP.add
            )
            nc.vector.tensor_copy(out=corrB[:, 1:2], in_=TS[:, rpp - 1 : rpp])
            winT = pp.tile([P, pitch], F32, tag="winT")
            nc.gpsimd.local_scatter(
                out_ap=winT[:].bitcast(U16), data_ap=corrB[:].bitcast(U16),
                idxs_ap=pidxT16[:, 0:4],
                channels=P, num_elems=2 * pitch, num_idxs=4,
            )
            nc.vector.tensor_tensor(out=winA[:], in0=winA[:], in1=winT[:], op=OP.add)
            win_fold(winA, accA, 0)
            nc.vector.tensor_tensor(out=meanb, in0=accA, in1=rec[:], op=OP.mult)
            nc.sync.dma_start(
                out=band_ext.ap().rearrange("(p k) -> p k", p=P), in_=bandout[:]
            )

    nc.finalize()
    return nc


_NC_CACHE: dict = {}


def _get_nc(*key):
    if key not in _NC_CACHE:
        _NC_CACHE[key] = build_nc(*key)
    return _NC_CACHE[key]


def kernel(x: np.ndarray, index: np.ndarray) -> np.ndarray:
    n = x.shape[0]
    assert n % (N_CORES * P * ROW) == 0, n
    epc = n // N_CORES

    # cheap structural checks on row heads (the algorithm's contract)
    heads = np.ascontiguousarray(index[::ROW]).astype(np.int64)
    dhh = np.diff(heads)
    if dhh.min() < 0 or dhh.max() > 1:
        raise ValueError("row-head steps outside {0,1}; kernel contract violated")
    hc = heads.reshape(N_CORES, P, -1)
    rel = hc - hc[:, 0:1, 0:1]
    slot = rel - K * np.arange(P)[None, :, None] + OFS
    if slot.min() < 0 or slot.max() + 1 >= WIN:
        raise ValueError("alignment window overflow; adjust K/OFS")
    if rel.max() + 1 >= 16384:
        raise ValueError("relative segment id exceeds int16 range")
    base0s = hc[:, 0, 0].astype(np.int64)  # first segment of each core
    widths = np.diff(np.concatenate([base0s, [NSEG]]))
    if widths.min() < 2 or widths.max() > BAND:
        raise ValueError("band widths outside (2, BAND]; kernel contract violated")

    nc = _get_nc(epc)

    in_maps = []
    for c in range(N_CORES):
        xs = np.ascontiguousarray(x[c * epc : (c + 1) * epc], dtype=np.float32)
        ii = (index[c * epc : (c + 1) * epc] - base0s[c]).astype(np.int16)
        in_maps.append({"x": xs, "idx": ii})

    res = run_bass_kernel_spmd(
        nc, in_maps, core_ids=list(range(N_CORES)), trace=TRACE, **RUN_KWARGS
    )
    global LAST_RESULT
    LAST_RESULT = res

    # host gather/unshard: concatenate per-core bands; recombine seam segments
    out = np.zeros(NSEG, dtype=np.float32)
    sums, cnts, means = [], [], []
    for c in range(N_CORES):
        arr = np.asarray(res.results[c]["band"], dtype=np.float32).reshape(P, 3 * K)
        sums.append(arr[:, 0:K].ravel())
        cnts.append(arr[:, K : 2 * K].ravel())
        means.append(arr[:, 2 * K : 3 * K].ravel())
    for c in range(N_CORES):
        lo = int(base0s[c])
        hi = int(base0s[c + 1]) if c < N_CORES - 1 else NSEG
        out[lo:hi] = means[c][0 : hi - lo]
    for c in range(N_CORES - 1):
        s = int(base0s[c + 1])  # seam segment shared by cores c and c+1
        if s >= NSEG:
            continue
        d = s - int(base0s[c])
        tot = sums[c][d] + sums[c + 1][0]
        cnt = cnts[c][d] + cnts[c + 1][0]
        out[s] = tot / max(cnt, 1.0)
    return out


TRACE = False
RUN_KWARGS: dict = {}
LAST_RESULT = None
